# revision 7
# baseline (speedup 1.0000x reference)
"""Causal self-attention (B=4, T=2048, C=1024, H=16, D=64) on 8 TRN2 NeuronCores.

Sharding: tensor-parallel over heads — each core owns 2 of the 16 heads.
Per core:
  qkv^T = W_pack.T @ x^T        (x^T streamed, W stationary; q/k/v each [2D, BT])
  S^T   = k_h^T.T @ q_h^T       (per batch, causal blocks only; 2 heads
                                 row-packed concurrently on the PE array)
  P^T   = exp(S^T/sqrt(D))      (no max-subtraction: logits are O(5))
  yu^T  = [v_h | 1].T @ P^T     (ones column accumulates the softmax denom)
  y^T   = yu^T * (1/denom)      (denom -> exp(-ln(d)) on ACT, replicated)
  out_p = y^T.T @ W_proj_rows   (partial over this core's head-rows)
Host: out = sum over cores of out_p.
"""

import sys

sys.path.insert(0, "/opt/trn_rl_repo")

import numpy as np
import ml_dtypes

import concourse.bass as bass
import concourse.bacc as bacc
import concourse.mybir as mybir
import concourse.tile as tile
from concourse.bass_utils import run_bass_kernel_spmd

BF16 = mybir.dt.bfloat16
F32 = mybir.dt.float32
AF = mybir.ActivationFunctionType

N_CORES = 8
N_HEAD = 16
N_EMBD = 1024
HEAD_DIM = N_EMBD // N_HEAD


class Cfg:
    def __init__(self, B=4, T=2048, C=1024, D=64, CH=512, TG=1024):
        self.B, self.T, self.C, self.D, self.CH, self.TG = B, T, C, D, CH, TG
        self.BT = B * T
        self.n_ct = C // 128          # contraction tiles for qkv
        self.nt = T // 128            # 128-row t-tiles per batch
        self.ncw = T // CH            # tq chunks per batch
        self.r = CH // 128            # t-tiles per chunk
        self.n_tg = self.BT // TG     # t-groups for qkv phase
        self.nchp = TG // CH          # chunks per t-group
        assert C % 128 == 0 and T % CH == 0 and CH % 128 == 0 and self.BT % TG == 0
        assert TG % CH == 0 and D == 64


def build(cfg: Cfg) -> bacc.Bacc:
    B, T, C, D, CH, TG = cfg.B, cfg.T, cfg.C, cfg.D, cfg.CH, cfg.TG
    BT, n_ct, nt, ncw, r = cfg.BT, cfg.n_ct, cfg.nt, cfg.ncw, cfg.r
    sm_scale = 1.0 / float(np.sqrt(D))

    nc = bacc.Bacc("TRN2", target_bir_lowering=False, debug=False,
                   num_devices=N_CORES)

    xT_d = nc.dram_tensor("xT", [C, BT], BF16, kind="ExternalInput")
    wq_d = nc.dram_tensor("wq", [128, n_ct * 128], BF16, kind="ExternalInput")
    wk_d = nc.dram_tensor("wk", [128, n_ct * 128], BF16, kind="ExternalInput")
    wv_d = nc.dram_tensor("wv", [128, n_ct * 128], BF16, kind="ExternalInput")
    wp_d = nc.dram_tensor("wp", [128, C], BF16, kind="ExternalInput")
    tri_d = nc.dram_tensor("tri", [128, 128], BF16, kind="ExternalInput")
    idn_d = nc.dram_tensor("idn", [128, 128], BF16, kind="ExternalInput")
    out_d = nc.dram_tensor("outp", [BT, C], F32, kind="ExternalOutput")

    with tile.TileContext(nc) as tc:
        with (
            tc.tile_pool(name="persist", bufs=1) as persist,
            tc.tile_pool(name="xt", bufs=2 * n_ct) as xt_pool,
            tc.tile_pool(name="pp", bufs=3) as p_pool,
            tc.tile_pool(name="vaug", bufs=2) as vaug_pool,
            tc.tile_pool(name="small", bufs=4) as small_pool,
            tc.tile_pool(name="rep", bufs=3) as rep_pool,
            tc.tile_pool(name="tmp1", bufs=3) as tmp_pool,
            tc.tile_pool(name="ps_mm", bufs=2, space="PSUM") as ps_mm,
            tc.tile_pool(name="ps_tr", bufs=1, space="PSUM") as ps_tr,
            tc.tile_pool(name="ps_s0", bufs=2, space="PSUM") as ps_s0,
            tc.tile_pool(name="ps_s1", bufs=1, space="PSUM") as ps_s1,
            tc.tile_pool(name="ps_yu0", bufs=1, space="PSUM") as ps_yu0,
            tc.tile_pool(name="ps_yu1", bufs=1, space="PSUM") as ps_yu1,
        ):
            # ---- persistent SBUF tensors -------------------------------
            qT = persist.tile([128, BT], BF16, tag="qT")
            kT = persist.tile([128, BT], BF16, tag="kT")
            vT = persist.tile([128, BT], BF16, tag="vT")
            yuT = persist.tile([128, BT], BF16, tag="yuT")
            wq_sb = persist.tile([128, n_ct * 128], BF16, tag="wq")
            wk_sb = persist.tile([128, n_ct * 128], BF16, tag="wk")
            wv_sb = persist.tile([128, n_ct * 128], BF16, tag="wv")
            wp_sb = persist.tile([128, C], BF16, tag="wp")
            tri_sb = persist.tile([128, 128], BF16, tag="tri")
            idn_sb = persist.tile([128, 128], BF16, tag="idn")
            nc.sync.dma_start(wq_sb[:], wq_d[:])
            nc.sync.dma_start(wk_sb[:], wk_d[:])
            nc.sync.dma_start(wv_sb[:], wv_d[:])
            nc.sync.dma_start(wp_sb[:], wp_d[:])
            nc.sync.dma_start(tri_sb[:], tri_d[:])
            nc.sync.dma_start(idn_sb[:], idn_d[:])

            # ---- phase 1: qkv^T = W.T @ x^T ----------------------------
            for g in range(cfg.n_tg):
                xts = []
                for ci in range(n_ct):
                    xt = xt_pool.tile([128, TG], BF16, tag="xt")
                    nc.sync.dma_start(
                        xt[:], xT_d[128 * ci:128 * (ci + 1),
                                    g * TG:(g + 1) * TG])
                    xts.append(xt)
                for wsb, dst in ((wq_sb, qT), (wk_sb, kT), (wv_sb, vT)):
                    for ch in range(cfg.nchp):
                        ps = ps_mm.tile([128, CH], F32, tag="mm")
                        for ci in range(n_ct):
                            nc.tensor.matmul(
                                ps[:],
                                wsb[:, 128 * ci:128 * (ci + 1)],
                                xts[ci][:, ch * CH:(ch + 1) * CH],
                                start=(ci == 0), stop=(ci == n_ct - 1))
                        nc.vector.tensor_copy(
                            dst[:, g * TG + ch * CH:g * TG + (ch + 1) * CH],
                            ps[:])

            # ---- phases 2+3 per batch: v_aug, attention ----------------
            for b in range(B):
                t0 = b * T
                # v natural layout + ones column, per head
                vaugs = []
                for h in (0, 1):
                    va = vaug_pool.tile([128, nt * 65], BF16, tag=f"vaug{h}")
                    ones_cols = va[:].rearrange("p (i c) -> p i c", c=65)[:, :, 64]
                    nc.vector.memset(ones_cols, 1.0)
                    vaugs.append(va)
                for i in range(nt):
                    for h in (0, 1):
                        pt = ps_tr.tile([128, 64], BF16, tag="tr")
                        nc.tensor.transpose(
                            pt[:],
                            vT[64 * h:64 * h + 64, t0 + 128 * i:t0 + 128 * (i + 1)],
                            idn_sb[64 * h:64 * h + 64, 64 * h:64 * h + 64])
                        nc.vector.tensor_copy(
                            vaugs[h][:, 65 * i:65 * i + 64], pt[:])

                for j in range(ncw):
                    yub = [ps_yu0.tile([65, CH], F32, tag="yu0", name="yu0"),
                           ps_yu1.tile([65, CH], F32, tag="yu1", name="yu1")]
                    i_max = r * (j + 1) - 1
                    for i in range(r * (j + 1)):
                        c0 = 128 * (i - r * j) if i >= r * j else 0
                        w = CH - c0
                        for h, ps_s in ((0, ps_s0), (1, ps_s1)):
                            ss = ps_s.tile([128, w], F32, tag=f"s{h}")
                            nc.tensor.matmul(
                                ss[:],
                                kT[64 * h:64 * h + 64,
                                   t0 + 128 * i:t0 + 128 * (i + 1)],
                                qT[64 * h:64 * h + 64,
                                   t0 + CH * j + c0:t0 + CH * (j + 1)],
                                start=True, stop=True)
                            pt_ = p_pool.tile([128, w], BF16, tag=f"p{h}")
                            nc.scalar.activation(pt_[:], ss[:], AF.Exp,
                                                 scale=sm_scale)
                            if i >= r * j:
                                nc.vector.tensor_mul(
                                    pt_[:, 0:128], pt_[:, 0:128], tri_sb[:])
                            nc.tensor.matmul(
                                yub[h][:, c0:CH],
                                vaugs[h][:, 65 * i:65 * i + 65],
                                pt_[:],
                                start=(i == 0), stop=(i == i_max))
                    # normalize: y^T = yu^T * exp(-ln(denom))
                    for h in (0, 1):
                        lg = small_pool.tile([65, CH], F32, tag="lg")
                        nc.scalar.activation(lg[64:65, :], yub[h][64:65, :],
                                             AF.Ln)
                        rec = small_pool.tile([65, CH], F32, tag="rec")
                        nc.scalar.activation(rec[64:65, :], lg[64:65, :],
                                             AF.Exp, scale=-1.0)
                        rec0 = small_pool.tile([1, CH], F32, tag="rec0")
                        nc.sync.dma_start(rec0[:], rec[64:65, :])
                        rep = rep_pool.tile([64, CH], F32, tag="rep")
                        nc.gpsimd.partition_broadcast(rep[:], rec0[0:1, :])
                        cols = slice(t0 + CH * j, t0 + CH * (j + 1))
                        if h == 0:
                            nc.vector.tensor_mul(
                                yuT[0:64, cols], yub[0][0:64, :], rep[:])
                        else:
                            tm = tmp_pool.tile([64, CH], BF16, tag="tmp1")
                            nc.vector.tensor_mul(tm[:], yub[1][0:64, :], rep[:])
                            nc.sync.dma_start(yuT[64:128, cols], tm[:])

            # ---- phase 4: out_p = y^T.T @ W_proj_rows ------------------
            for m in range(BT // 128):
                for ch in range(C // CH):
                    po = ps_mm.tile([128, CH], F32, tag="mm")
                    nc.tensor.matmul(
                        po[:],
                        yuT[:, 128 * m:128 * (m + 1)],
                        wp_sb[:, ch * CH:(ch + 1) * CH],
                        start=True, stop=True)
                    ob = tmp_pool.tile([128, CH], F32, tag="ob", bufs=3)
                    nc.vector.tensor_copy(ob[:], po[:])
                    nc.sync.dma_start(
                        out_d[128 * m:128 * (m + 1), ch * CH:(ch + 1) * CH],
                        ob[:])

    nc.compile()
    return nc


def host_inputs(cfg: Cfg, x, W_attn, W_proj, h0, hpc=2):
    """Per-core input dict for the core owning heads [h0, h0+hpc)."""
    C, D = cfg.C, cfg.D
    assert hpc * D == 128
    bf = ml_dtypes.bfloat16

    def wpack(Wcols):  # [C, 128] -> [128, n_ct*128] (c-tile minor)
        return np.ascontiguousarray(
            Wcols.reshape(cfg.n_ct, 128, 128).transpose(1, 0, 2)
            .reshape(128, cfg.n_ct * 128)).astype(bf)

    cols = np.concatenate([np.arange(h * D, (h + 1) * D)
                           for h in range(h0, h0 + hpc)])
    return {
        "wq": wpack(W_attn[:, cols]),
        "wk": wpack(W_attn[:, C + cols]),
        "wv": wpack(W_attn[:, 2 * C + cols]),
        "wp": np.ascontiguousarray(W_proj[cols, :]).astype(bf),
        "tri": np.triu(np.ones((128, 128))).astype(bf),
        "idn": np.eye(128).astype(bf),
    }


_NC_CACHE = {}


def kernel(x, W_attn, W_proj):
    x = np.asarray(x)
    W_attn = np.asarray(W_attn)
    W_proj = np.asarray(W_proj)
    B, T, C = x.shape
    cfg = Cfg(B=B, T=T, C=C)
    key = (B, T, C)
    if key not in _NC_CACHE:
        _NC_CACHE[key] = build(cfg)
    nc = _NC_CACHE[key]

    xT = np.ascontiguousarray(x.reshape(cfg.BT, C).T).astype(ml_dtypes.bfloat16)
    in_maps = []
    for core in range(N_CORES):
        m = host_inputs(cfg, x, W_attn, W_proj, h0=2 * core)
        m["xT"] = xT
        in_maps.append(m)

    res = run_bass_kernel_spmd(nc, in_maps, core_ids=list(range(N_CORES)))
    out = np.zeros((cfg.BT, C), dtype=np.float64)
    for core in range(N_CORES):
        out += res.results[core]["outp"].astype(np.float64)
    return out.astype(np.float32).reshape(B, T, C)


# revision 16
# speedup vs baseline: 1.0023x; 1.0023x over previous
"""Causal self-attention (B=4, T=2048, C=1024, H=16, D=64) on 8 TRN2 NeuronCores.

Sharding: tensor-parallel over heads — each core owns 2 of the 16 heads.
Per core:
  qkv^T = W_pack.T @ x^T        (x^T streamed, W stationary; q/k/v each [2D, BT])
  S^T   = k_h^T.T @ q_h^T       (per batch, causal blocks only; 2 heads
                                 row-packed concurrently on the PE array)
  P^T   = exp(S^T/sqrt(D))      (no max-subtraction: logits are O(5))
  yu^T  = [v_h | 1].T @ P^T     (ones column accumulates the softmax denom)
  y^T   = yu^T * (1/denom)      (denom -> exp(-ln(d)) on ACT, replicated)
  out_p = y^T.T @ W_proj_rows   (partial over this core's head-rows)
Host: out = sum over cores of out_p.
"""

import sys

sys.path.insert(0, "/opt/trn_rl_repo")

import numpy as np
import ml_dtypes

import concourse.bass as bass
import concourse.bacc as bacc
import concourse.mybir as mybir
import concourse.tile as tile
from concourse.bass_utils import run_bass_kernel_spmd

BF16 = mybir.dt.bfloat16
F32 = mybir.dt.float32
AF = mybir.ActivationFunctionType

N_CORES = 8
N_HEAD = 16
N_EMBD = 1024
HEAD_DIM = N_EMBD // N_HEAD


class Cfg:
    def __init__(self, B=4, T=2048, C=1024, D=64, CH=512, TG=1024):
        self.B, self.T, self.C, self.D, self.CH, self.TG = B, T, C, D, CH, TG
        self.BT = B * T
        self.n_ct = C // 128          # contraction tiles for qkv
        self.nt = T // 128            # 128-row t-tiles per batch
        self.ncw = T // CH            # tq chunks per batch
        self.r = CH // 128            # t-tiles per chunk
        self.n_tg = self.BT // TG     # t-groups for qkv phase
        self.nchp = TG // CH          # chunks per t-group
        assert C % 128 == 0 and T % CH == 0 and CH % 128 == 0 and self.BT % TG == 0
        assert TG % CH == 0 and D == 64


def build(cfg: Cfg) -> bacc.Bacc:
    B, T, C, D, CH, TG = cfg.B, cfg.T, cfg.C, cfg.D, cfg.CH, cfg.TG
    BT, n_ct, nt, ncw, r = cfg.BT, cfg.n_ct, cfg.nt, cfg.ncw, cfg.r
    sm_scale = 1.0 / float(np.sqrt(D))

    nc = bacc.Bacc("TRN2", target_bir_lowering=False, debug=False,
                   num_devices=N_CORES)

    xT_d = nc.dram_tensor("xT", [C, BT], BF16, kind="ExternalInput")
    wq_d = nc.dram_tensor("wq", [128, n_ct * 128], BF16, kind="ExternalInput")
    wk_d = nc.dram_tensor("wk", [128, n_ct * 128], BF16, kind="ExternalInput")
    wv_d = nc.dram_tensor("wv", [128, n_ct * 128], BF16, kind="ExternalInput")
    wp_d = nc.dram_tensor("wp", [128, C], BF16, kind="ExternalInput")
    tri_d = nc.dram_tensor("tri", [128, 128], BF16, kind="ExternalInput")
    idn_d = nc.dram_tensor("idn", [128, 128], BF16, kind="ExternalInput")
    idf_d = nc.dram_tensor("idf", [128, 128], F32, kind="ExternalInput")
    out_d = nc.dram_tensor("outp", [BT, C], F32, kind="ExternalOutput")

    with tile.TileContext(nc) as tc:
        with (
            tc.tile_pool(name="persist", bufs=1) as persist,
            tc.tile_pool(name="xt", bufs=2 * n_ct) as xt_pool,
            tc.tile_pool(name="pp", bufs=3) as p_pool,
            tc.tile_pool(name="vaug", bufs=2) as vaug_pool,
            tc.tile_pool(name="small", bufs=4) as small_pool,
            tc.tile_pool(name="rep", bufs=3) as rep_pool,
            tc.tile_pool(name="tmp1", bufs=3) as tmp_pool,
            tc.tile_pool(name="ps_mm", bufs=2, space="PSUM") as ps_mm,
            tc.tile_pool(name="ps_s", bufs=2, space="PSUM") as ps_s,
            tc.tile_pool(name="ps_yu0", bufs=1, space="PSUM") as ps_yu0,
            tc.tile_pool(name="ps_yu1", bufs=1, space="PSUM") as ps_yu1,
        ):
            # ---- persistent SBUF tensors -------------------------------
            qT = persist.tile([128, BT], BF16, tag="qT")
            kT = persist.tile([128, BT], BF16, tag="kT")
            vT = persist.tile([128, BT], BF16, tag="vT")
            yuT = persist.tile([128, BT], BF16, tag="yuT")
            wq_sb = persist.tile([128, n_ct * 128], BF16, tag="wq")
            wk_sb = persist.tile([128, n_ct * 128], BF16, tag="wk")
            wv_sb = persist.tile([128, n_ct * 128], BF16, tag="wv")
            wp_sb = persist.tile([128, C], BF16, tag="wp")
            tri_sb = persist.tile([128, 128], BF16, tag="tri")
            idn_sb = persist.tile([128, 128], BF16, tag="idn")
            idf_sb = persist.tile([128, 128], F32, tag="idf")
            nc.sync.dma_start(idf_sb[:], idf_d[:])
            nc.sync.dma_start(wq_sb[:], wq_d[:])
            nc.sync.dma_start(wk_sb[:], wk_d[:])
            nc.sync.dma_start(wv_sb[:], wv_d[:])
            nc.sync.dma_start(wp_sb[:], wp_d[:])
            nc.sync.dma_start(tri_sb[:], tri_d[:])
            nc.sync.dma_start(idn_sb[:], idn_d[:])

            # ---- phase 1: qkv^T = W.T @ x^T ----------------------------
            for g in range(cfg.n_tg):
                xts = []
                for ci in range(n_ct):
                    xt = xt_pool.tile([128, TG], BF16, tag="xt")
                    nc.sync.dma_start(
                        xt[:], xT_d[128 * ci:128 * (ci + 1),
                                    g * TG:(g + 1) * TG])
                    xts.append(xt)
                for wsb, dst in ((wq_sb, qT), (wk_sb, kT), (wv_sb, vT)):
                    for ch in range(cfg.nchp):
                        ps = ps_mm.tile([128, CH], F32, tag="mm")
                        for ci in range(n_ct):
                            nc.tensor.matmul(
                                ps[:],
                                wsb[:, 128 * ci:128 * (ci + 1)],
                                xts[ci][:, ch * CH:(ch + 1) * CH],
                                start=(ci == 0), stop=(ci == n_ct - 1))
                        nc.vector.tensor_copy(
                            dst[:, g * TG + ch * CH:g * TG + (ch + 1) * CH],
                            ps[:])

            # ---- phases 2+3 per batch: v_aug, attention ----------------
            for b in range(B):
                t0 = b * T
                # v natural layout + ones column, per head
                vaugs = []
                for h in (0, 1):
                    va = vaug_pool.tile([128, nt * 65], BF16, tag=f"vaug{h}")
                    ones_cols = va[:].rearrange("p (i c) -> p i c", c=65)[:, :, 64]
                    nc.vector.memset(ones_cols, 1.0)
                    vaugs.append(va)
                for i in range(nt):
                    for h in (0, 1):
                        pt = ps_mm.tile([128, 64], BF16, tag="mm", name="pt")
                        nc.tensor.transpose(
                            pt[:],
                            vT[64 * h:64 * h + 64, t0 + 128 * i:t0 + 128 * (i + 1)],
                            idn_sb[64 * h:64 * h + 64, 64 * h:64 * h + 64])
                        nc.vector.tensor_copy(
                            vaugs[h][:, 65 * i:65 * i + 64], pt[:])

                for j in range(ncw):
                    yub = [ps_yu0.tile([65, CH], F32, tag="yu0", name="yu0"),
                           ps_yu1.tile([65, CH], F32, tag="yu1", name="yu1")]
                    i_max = r * (j + 1) - 1
                    for i in range(r * (j + 1)):
                        c0 = 128 * (i - r * j) if i >= r * j else 0
                        w = CH - c0
                        ss = ps_s.tile([128, 2 * CH], F32, tag="s", name="ss")
                        for h in (0, 1):
                            nc.tensor.matmul(
                                ss[:, h * CH:h * CH + w],
                                kT[64 * h:64 * h + 64,
                                   t0 + 128 * i:t0 + 128 * (i + 1)],
                                qT[64 * h:64 * h + 64,
                                   t0 + CH * j + c0:t0 + CH * (j + 1)],
                                start=True, stop=True)
                        pt_ = p_pool.tile([128, 2 * w], BF16, tag="p",
                                          name="pt_")
                        if w == CH:
                            nc.scalar.activation(pt_[:], ss[:], AF.Exp,
                                                 scale=sm_scale)
                        else:
                            for h in (0, 1):
                                nc.scalar.activation(
                                    pt_[:, h * w:(h + 1) * w],
                                    ss[:, h * CH:h * CH + w], AF.Exp,
                                    scale=sm_scale)
                        if i >= r * j:
                            for h in (0, 1):
                                nc.vector.tensor_mul(
                                    pt_[:, h * w:h * w + 128],
                                    pt_[:, h * w:h * w + 128], tri_sb[:])
                        for h in (0, 1):
                            nc.tensor.matmul(
                                yub[h][:, c0:CH],
                                vaugs[h][:, 65 * i:65 * i + 65],
                                pt_[:, h * w:(h + 1) * w],
                                start=(i == 0), stop=(i == i_max))
                    # normalize: y^T = yu^T * recip(denom).  The denom rows
                    # live at PSUM partition 64 (free-axis layout); recip on
                    # DVE costs 8 cyc/free-elem, so transpose the rows into
                    # column layout with PE mini-transposes first, recip the
                    # [128, ...] column block, and transpose back.
                    nq = CH // 128
                    dens = []
                    for h in (0, 1):
                        den = small_pool.tile([65, CH], F32, tag=f"den{h}",
                                              name=f"den{h}")
                        nc.scalar.copy(den[64:65, :], yub[h][64:65, :])
                        dens.append(den)
                    dt = ps_mm.tile([128, 2 * nq], F32, tag="mm", name="dt")
                    for h in (0, 1):
                        for k in range(nq):
                            nc.tensor.transpose(
                                dt[:, nq * h + k:nq * h + k + 1],
                                dens[h][64:65, 128 * k:128 * (k + 1)],
                                idf_sb[64:65, 64:65])
                    rcol = small_pool.tile([128, 2 * nq], F32, tag="rcol")
                    nc.vector.reciprocal(rcol[:], dt[:])
                    for h in (0, 1):
                        rb = ps_mm.tile([1, CH], F32, tag="mm", name="rb")
                        for k in range(nq):
                            nc.tensor.transpose(
                                rb[0:1, 128 * k:128 * (k + 1)],
                                rcol[:, nq * h + k:nq * h + k + 1],
                                idf_sb[:, :])
                        rec0 = small_pool.tile([1, CH], F32, tag="rec0")
                        nc.scalar.copy(rec0[:], rb[:])
                        rep = rep_pool.tile([64, CH], F32, tag="rep")
                        nc.gpsimd.partition_broadcast(rep[:], rec0[0:1, :])
                        cols = slice(t0 + CH * j, t0 + CH * (j + 1))
                        if h == 0:
                            nc.vector.tensor_mul(
                                yuT[0:64, cols], yub[0][0:64, :], rep[:])
                        else:
                            tm = tmp_pool.tile([64, CH], BF16, tag="tmp1")
                            nc.vector.tensor_mul(tm[:], yub[1][0:64, :], rep[:])
                            nc.sync.dma_start(yuT[64:128, cols], tm[:])

            # ---- phase 4: out_p = y^T.T @ W_proj_rows ------------------
            for m in range(BT // 128):
                for ch in range(C // CH):
                    po = ps_mm.tile([128, CH], F32, tag="mm")
                    nc.tensor.matmul(
                        po[:],
                        yuT[:, 128 * m:128 * (m + 1)],
                        wp_sb[:, ch * CH:(ch + 1) * CH],
                        start=True, stop=True)
                    ob = tmp_pool.tile([128, CH], F32, tag="ob", bufs=3)
                    nc.vector.tensor_copy(ob[:], po[:])
                    nc.sync.dma_start(
                        out_d[128 * m:128 * (m + 1), ch * CH:(ch + 1) * CH],
                        ob[:])

    nc.compile()
    return nc


def host_inputs(cfg: Cfg, x, W_attn, W_proj, h0, hpc=2):
    """Per-core input dict for the core owning heads [h0, h0+hpc)."""
    C, D = cfg.C, cfg.D
    assert hpc * D == 128
    bf = ml_dtypes.bfloat16

    def wpack(Wcols):  # [C, 128] -> [128, n_ct*128] (c-tile minor)
        return np.ascontiguousarray(
            Wcols.reshape(cfg.n_ct, 128, 128).transpose(1, 0, 2)
            .reshape(128, cfg.n_ct * 128)).astype(bf)

    cols = np.concatenate([np.arange(h * D, (h + 1) * D)
                           for h in range(h0, h0 + hpc)])
    return {
        "wq": wpack(W_attn[:, cols]),
        "wk": wpack(W_attn[:, C + cols]),
        "wv": wpack(W_attn[:, 2 * C + cols]),
        "wp": np.ascontiguousarray(W_proj[cols, :]).astype(bf),
        "tri": np.triu(np.ones((128, 128))).astype(bf),
        "idn": np.eye(128).astype(bf),
        "idf": np.eye(128, dtype=np.float32),
    }


_NC_CACHE = {}


def kernel(x, W_attn, W_proj):
    x = np.asarray(x)
    W_attn = np.asarray(W_attn)
    W_proj = np.asarray(W_proj)
    B, T, C = x.shape
    cfg = Cfg(B=B, T=T, C=C)
    key = (B, T, C)
    if key not in _NC_CACHE:
        _NC_CACHE[key] = build(cfg)
    nc = _NC_CACHE[key]

    xT = np.ascontiguousarray(x.reshape(cfg.BT, C).T).astype(ml_dtypes.bfloat16)
    in_maps = []
    for core in range(N_CORES):
        m = host_inputs(cfg, x, W_attn, W_proj, h0=2 * core)
        m["xT"] = xT
        in_maps.append(m)

    res = run_bass_kernel_spmd(nc, in_maps, core_ids=list(range(N_CORES)))
    out = np.zeros((cfg.BT, C), dtype=np.float64)
    for core in range(N_CORES):
        out += res.results[core]["outp"].astype(np.float64)
    return out.astype(np.float32).reshape(B, T, C)


# revision 19
# speedup vs baseline: 1.2026x; 1.1998x over previous
"""Causal self-attention (B=4, T=2048, C=1024, H=16, D=64) on 8 TRN2 NeuronCores.

Sharding: tensor-parallel over heads — each core owns 2 of the 16 heads.
Per core:
  qkv^T = W_pack.T @ x^T        (x^T streamed, W stationary; q/k/v each [2D, BT])
  S^T   = k_h^T.T @ q_h^T       (per batch, causal blocks only; 2 heads
                                 row-packed concurrently on the PE array)
  P^T   = exp(S^T/sqrt(D))      (no max-subtraction: logits are O(5))
  yu^T  = [v_h | 1].T @ P^T     (ones column accumulates the softmax denom)
  y^T   = yu^T * (1/denom)      (recip via PE row<->col transposes + DVE)
  out_p = y^T.T @ W_proj_rows   (partial over this core's head-rows, fp16)
Host: out = sum over cores of out_p.
"""

import sys

sys.path.insert(0, "/opt/trn_rl_repo")

import numpy as np
import ml_dtypes

import concourse.bass as bass
import concourse.bacc as bacc
import concourse.mybir as mybir
import concourse.tile as tile
from concourse.bass_utils import run_bass_kernel_spmd

BF16 = mybir.dt.bfloat16
F16 = mybir.dt.float16
F32 = mybir.dt.float32
AF = mybir.ActivationFunctionType

N_CORES = 8
N_HEAD = 16
N_EMBD = 1024
HEAD_DIM = N_EMBD // N_HEAD


class Cfg:
    def __init__(self, B=4, T=2048, C=1024, D=64, CH=512, TG=1024):
        self.B, self.T, self.C, self.D, self.CH, self.TG = B, T, C, D, CH, TG
        self.BT = B * T
        self.n_ct = C // 128          # contraction tiles for qkv
        self.nt = T // 128            # 128-row t-tiles per batch
        self.ncw = T // CH            # tq chunks per batch
        self.r = CH // 128            # t-tiles per chunk
        self.ngb = T // TG            # t-groups per batch (qkv phase)
        self.nchp = TG // CH          # chunks per t-group
        assert C % 128 == 0 and T % CH == 0 and CH % 128 == 0 and T % TG == 0
        assert TG % CH == 0 and D == 64


def build(cfg: Cfg) -> bacc.Bacc:
    B, T, C, D, CH, TG = cfg.B, cfg.T, cfg.C, cfg.D, cfg.CH, cfg.TG
    BT, n_ct, nt, ncw, r = cfg.BT, cfg.n_ct, cfg.nt, cfg.ncw, cfg.r
    sm_scale = 1.0 / float(np.sqrt(D))
    nq = CH // 128

    nc = bacc.Bacc("TRN2", target_bir_lowering=False, debug=False,
                   num_devices=N_CORES)

    xT_d = nc.dram_tensor("xT", [C, BT], BF16, kind="ExternalInput")
    wq_d = nc.dram_tensor("wq", [128, n_ct * 128], BF16, kind="ExternalInput")
    wk_d = nc.dram_tensor("wk", [128, n_ct * 128], BF16, kind="ExternalInput")
    wv_d = nc.dram_tensor("wv", [128, n_ct * 128], BF16, kind="ExternalInput")
    wp_d = nc.dram_tensor("wp", [128, C], BF16, kind="ExternalInput")
    tri_d = nc.dram_tensor("tri", [128, 128], BF16, kind="ExternalInput")
    idn_d = nc.dram_tensor("idn", [128, 128], BF16, kind="ExternalInput")
    idf_d = nc.dram_tensor("idf", [128, 128], F32, kind="ExternalInput")
    out_d = nc.dram_tensor("outp", [BT, C], F16, kind="ExternalOutput")

    with tile.TileContext(nc) as tc:
        with (
            tc.tile_pool(name="persist", bufs=1) as persist,
            tc.tile_pool(name="xt", bufs=2 * n_ct) as xt_pool,
            tc.tile_pool(name="pp", bufs=4) as p_pool,
            tc.tile_pool(name="vaug", bufs=2) as vaug_pool,
            tc.tile_pool(name="small", bufs=4) as small_pool,
            tc.tile_pool(name="rep", bufs=3) as rep_pool,
            tc.tile_pool(name="tmp1", bufs=3) as tmp_pool,
            tc.tile_pool(name="ps_mm", bufs=2, space="PSUM") as ps_mm,
            tc.tile_pool(name="ps_s", bufs=2, space="PSUM") as ps_s,
            tc.tile_pool(name="ps_yu0", bufs=1, space="PSUM") as ps_yu0,
            tc.tile_pool(name="ps_yu1", bufs=1, space="PSUM") as ps_yu1,
        ):
            # ---- persistent SBUF tensors -------------------------------
            qTs = [persist.tile([128, T], BF16, tag=f"qT{b}", name=f"qT{b}")
                   for b in range(B)]
            kTs = [persist.tile([128, T], BF16, tag=f"kT{b}", name=f"kT{b}")
                   for b in range(B)]
            vTs = [persist.tile([128, T], BF16, tag=f"vT{b}", name=f"vT{b}")
                   for b in range(B)]
            yuTs = [persist.tile([128, T], BF16, tag=f"yuT{b}", name=f"yuT{b}")
                    for b in range(B)]
            wq_sb = persist.tile([128, n_ct * 128], BF16, tag="wq")
            wk_sb = persist.tile([128, n_ct * 128], BF16, tag="wk")
            wv_sb = persist.tile([128, n_ct * 128], BF16, tag="wv")
            wp_sb = persist.tile([128, C], BF16, tag="wp")
            tri_sb = persist.tile([128, 128], BF16, tag="tri")
            idn_sb = persist.tile([128, 128], BF16, tag="idn")
            idf_sb = persist.tile([128, 128], F32, tag="idf")
            nc.sync.dma_start(idf_sb[:], idf_d[:])
            nc.sync.dma_start(wq_sb[:], wq_d[:])
            nc.sync.dma_start(wk_sb[:], wk_d[:])
            nc.sync.dma_start(wv_sb[:], wv_d[:])
            nc.sync.dma_start(wp_sb[:], wp_d[:])
            nc.sync.dma_start(tri_sb[:], tri_d[:])
            nc.sync.dma_start(idn_sb[:], idn_d[:])

            # ---- phase 1: qkv^T = W.T @ x^T ----------------------------
            for b in range(B):
                for gl in range(cfg.ngb):
                    g0 = b * T + gl * TG       # global col base
                    l0 = gl * TG               # batch-local col base
                    xts = []
                    for ci in range(n_ct):
                        xt = xt_pool.tile([128, TG], BF16, tag="xt")
                        nc.sync.dma_start(
                            xt[:], xT_d[128 * ci:128 * (ci + 1),
                                        g0:g0 + TG])
                        xts.append(xt)
                    for wsb, dsts in ((wq_sb, qTs), (wk_sb, kTs),
                                      (wv_sb, vTs)):
                        for ch in range(cfg.nchp):
                            ps = ps_mm.tile([128, CH], F32, tag="mm",
                                            name="ps")
                            for ci in range(n_ct):
                                nc.tensor.matmul(
                                    ps[:],
                                    wsb[:, 128 * ci:128 * (ci + 1)],
                                    xts[ci][:, ch * CH:(ch + 1) * CH],
                                    start=(ci == 0), stop=(ci == n_ct - 1))
                            nc.vector.tensor_copy(
                                dsts[b][:, l0 + ch * CH:l0 + (ch + 1) * CH],
                                ps[:])

            # ---- phases 2+3 per batch: v_aug, attention ----------------
            for b in range(B):
                qT, kT, vT, yuT = qTs[b], kTs[b], vTs[b], yuTs[b]
                # v natural layout + ones column, per head
                vaugs = []
                for h in (0, 1):
                    va = vaug_pool.tile([128, nt * 65], BF16, tag=f"vaug{h}",
                                        name=f"va{h}")
                    ones_cols = va[:].rearrange("p (i c) -> p i c",
                                                c=65)[:, :, 64]
                    nc.vector.memset(ones_cols, 1.0)
                    vaugs.append(va)
                for i in range(nt):
                    for h in (0, 1):
                        pt = ps_mm.tile([128, 64], BF16, tag="mm", name="pt")
                        nc.tensor.transpose(
                            pt[:],
                            vT[64 * h:64 * h + 64, 128 * i:128 * (i + 1)],
                            idn_sb[64 * h:64 * h + 64, 64 * h:64 * h + 64])
                        nc.vector.tensor_copy(
                            vaugs[h][:, 65 * i:65 * i + 64], pt[:])

                for j in range(ncw):
                    yub = [ps_yu0.tile([65, CH], F32, tag="yu0", name="yu0"),
                           ps_yu1.tile([65, CH], F32, tag="yu1", name="yu1")]
                    i_max = r * (j + 1) - 1
                    for i in range(r * (j + 1)):
                        c0 = 128 * (i - r * j) if i >= r * j else 0
                        w = CH - c0
                        ss = ps_s.tile([128, 2 * CH], F32, tag="s", name="ss")
                        for h in (0, 1):
                            nc.tensor.matmul(
                                ss[:, h * CH:h * CH + w],
                                kT[64 * h:64 * h + 64,
                                   128 * i:128 * (i + 1)],
                                qT[64 * h:64 * h + 64,
                                   CH * j + c0:CH * (j + 1)],
                                start=True, stop=True)
                        pt_ = p_pool.tile([128, 2 * w], BF16, tag="p",
                                          name="pt_")
                        if w == CH:
                            nc.scalar.activation(pt_[:], ss[:], AF.Exp,
                                                 scale=sm_scale)
                        else:
                            for h in (0, 1):
                                nc.scalar.activation(
                                    pt_[:, h * w:(h + 1) * w],
                                    ss[:, h * CH:h * CH + w], AF.Exp,
                                    scale=sm_scale)
                        if i >= r * j:
                            for h in (0, 1):
                                nc.vector.tensor_mul(
                                    pt_[:, h * w:h * w + 128],
                                    pt_[:, h * w:h * w + 128], tri_sb[:])
                        for h in (0, 1):
                            nc.tensor.matmul(
                                yub[h][:, c0:CH],
                                vaugs[h][:, 65 * i:65 * i + 65],
                                pt_[:, h * w:(h + 1) * w],
                                start=(i == 0), stop=(i == i_max))
                    # normalize: y^T = yu^T * recip(denom).  denom rows sit
                    # at PSUM partition 64; PE-transpose them to column
                    # layout, exact-recip on DVE (cheap on the free axis),
                    # PE-transpose back, broadcast across partitions.
                    dens = []
                    for h in (0, 1):
                        den = small_pool.tile([65, CH], F32, tag=f"den{h}",
                                              name=f"den{h}")
                        nc.vector.tensor_copy(den[64:65, :],
                                              yub[h][64:65, :])
                        dens.append(den)
                    dt = ps_mm.tile([128, 2 * nq], F32, tag="mm", name="dt")
                    for h in (0, 1):
                        for k in range(nq):
                            nc.tensor.transpose(
                                dt[:, nq * h + k:nq * h + k + 1],
                                dens[h][64:65, 128 * k:128 * (k + 1)],
                                idf_sb[64:65, 64:65])
                    rcol = small_pool.tile([128, 2 * nq], F32, tag="rcol")
                    nc.vector.reciprocal(rcol[:], dt[:])
                    rb = ps_mm.tile([2 * nq, 128], F32, tag="mm", name="rb")
                    nc.tensor.transpose(rb[:], rcol[:], idf_sb[:, :])
                    rbs = small_pool.tile([2 * nq, 128], F32, tag="rbs")
                    nc.vector.tensor_copy(rbs[:], rb[:])
                    for h in (0, 1):
                        rec0 = small_pool.tile([1, CH], F32, tag="rec0",
                                               name="rec0")
                        nc.sync.dma_start(
                            rec0[0:1, :], rbs[nq * h:nq * (h + 1), :])
                        rep = rep_pool.tile([64, CH], F32, tag="rep",
                                            name="rep")
                        nc.gpsimd.partition_broadcast(rep[:], rec0[0:1, :])
                        cols = slice(CH * j, CH * (j + 1))
                        if h == 0:
                            nc.vector.tensor_mul(
                                yuT[0:64, cols], yub[0][0:64, :], rep[:])
                        else:
                            tm = tmp_pool.tile([64, CH], BF16, tag="tmp1")
                            nc.vector.tensor_mul(tm[:], yub[1][0:64, :],
                                                 rep[:])
                            nc.sync.dma_start(yuT[64:128, cols], tm[:])

            # ---- phase 4: out_p = y^T.T @ W_proj_rows ------------------
            for b in range(B):
                for m in range(nt):
                    po = ps_s.tile([128, 2 * CH], F32, tag="s", name="po")
                    for ch in range(C // CH):
                        nc.tensor.matmul(
                            po[:, ch * CH:(ch + 1) * CH],
                            yuTs[b][:, 128 * m:128 * (m + 1)],
                            wp_sb[:, ch * CH:(ch + 1) * CH],
                            start=True, stop=True)
                    ob = tmp_pool.tile([128, C], F16, tag="ob", bufs=3,
                                       name="ob")
                    nc.scalar.copy(ob[:], po[:, 0:C])
                    nc.sync.dma_start(
                        out_d[b * T + 128 * m:b * T + 128 * (m + 1), :],
                        ob[:])

    nc.compile()
    return nc


def host_inputs(cfg: Cfg, x, W_attn, W_proj, h0, hpc=2):
    """Per-core input dict for the core owning heads [h0, h0+hpc)."""
    C, D = cfg.C, cfg.D
    assert hpc * D == 128
    bf = ml_dtypes.bfloat16

    def wpack(Wcols):  # [C, 128] -> [128, n_ct*128] (c-tile minor)
        return np.ascontiguousarray(
            Wcols.reshape(cfg.n_ct, 128, 128).transpose(1, 0, 2)
            .reshape(128, cfg.n_ct * 128)).astype(bf)

    cols = np.concatenate([np.arange(h * D, (h + 1) * D)
                           for h in range(h0, h0 + hpc)])
    return {
        "wq": wpack(W_attn[:, cols]),
        "wk": wpack(W_attn[:, C + cols]),
        "wv": wpack(W_attn[:, 2 * C + cols]),
        "wp": np.ascontiguousarray(W_proj[cols, :]).astype(bf),
        "tri": np.triu(np.ones((128, 128))).astype(bf),
        "idn": np.eye(128).astype(bf),
        "idf": np.eye(128, dtype=np.float32),
    }


_NC_CACHE = {}


def kernel(x, W_attn, W_proj):
    x = np.asarray(x)
    W_attn = np.asarray(W_attn)
    W_proj = np.asarray(W_proj)
    B, T, C = x.shape
    cfg = Cfg(B=B, T=T, C=C)
    key = (B, T, C)
    if key not in _NC_CACHE:
        _NC_CACHE[key] = build(cfg)
    nc = _NC_CACHE[key]

    xT = np.ascontiguousarray(x.reshape(cfg.BT, C).T).astype(ml_dtypes.bfloat16)
    in_maps = []
    for core in range(N_CORES):
        m = host_inputs(cfg, x, W_attn, W_proj, h0=2 * core)
        m["xT"] = xT
        in_maps.append(m)

    res = run_bass_kernel_spmd(nc, in_maps, core_ids=list(range(N_CORES)))
    out = np.zeros((cfg.BT, C), dtype=np.float64)
    for core in range(N_CORES):
        out += res.results[core]["outp"].astype(np.float64)
    return out.astype(np.float32).reshape(B, T, C)


# revision 21
# speedup vs baseline: 1.4222x; 1.1826x over previous
"""Causal self-attention (B=4, T=2048, C=1024, H=16, D=64) on 8 TRN2 NeuronCores.

Sharding: tensor-parallel over heads — each core owns 2 of the 16 heads.
Per core:
  qkv^T = W_pack.T @ x^T        (x^T streamed, W stationary; q/k/v each [2D, BT])
  S^T   = k_h^T.T @ q_h^T       (per batch, causal blocks only; 2 heads
                                 row-packed concurrently on the PE array)
  P^T   = exp(S^T/sqrt(D))      (no max-subtraction: logits are O(5))
  yu^T  = [v_h | 1].T @ P^T     (ones column accumulates the softmax denom)
  y^T   = yu^T * (1/denom)      (recip via PE row<->col transposes + DVE)
  out_p = y^T.T @ W_proj_rows   (partial over this core's head-rows, fp16)
Host: out = sum over cores of out_p.
"""

import sys

sys.path.insert(0, "/opt/trn_rl_repo")

import numpy as np
import ml_dtypes

import concourse.bass as bass
import concourse.bacc as bacc
import concourse.mybir as mybir
import concourse.tile as tile
from concourse.bass_utils import run_bass_kernel_spmd

BF16 = mybir.dt.bfloat16
F16 = mybir.dt.float16
F32 = mybir.dt.float32
AF = mybir.ActivationFunctionType

N_CORES = 8
N_HEAD = 16
N_EMBD = 1024
HEAD_DIM = N_EMBD // N_HEAD


class Cfg:
    def __init__(self, B=4, T=2048, C=1024, D=64, CH=512, TG=1024):
        self.B, self.T, self.C, self.D, self.CH, self.TG = B, T, C, D, CH, TG
        self.BT = B * T
        self.n_ct = C // 128          # contraction tiles for qkv
        self.nt = T // 128            # 128-row t-tiles per batch
        self.ncw = T // CH            # tq chunks per batch
        self.r = CH // 128            # t-tiles per chunk
        self.ngb = T // TG            # t-groups per batch (qkv phase)
        self.nchp = TG // CH          # chunks per t-group
        assert C % 128 == 0 and T % CH == 0 and CH % 128 == 0 and T % TG == 0
        assert TG % CH == 0 and D == 64


def build(cfg: Cfg) -> bacc.Bacc:
    B, T, C, D, CH, TG = cfg.B, cfg.T, cfg.C, cfg.D, cfg.CH, cfg.TG
    BT, n_ct, nt, ncw, r = cfg.BT, cfg.n_ct, cfg.nt, cfg.ncw, cfg.r
    sm_scale = 1.0 / float(np.sqrt(D))
    nq = CH // 128

    nc = bacc.Bacc("TRN2", target_bir_lowering=False, debug=False,
                   num_devices=N_CORES)

    xT_d = nc.dram_tensor("xT", [C, BT], BF16, kind="ExternalInput")
    wq_d = nc.dram_tensor("wq", [128, n_ct * 128], BF16, kind="ExternalInput")
    wk_d = nc.dram_tensor("wk", [128, n_ct * 128], BF16, kind="ExternalInput")
    wv_d = nc.dram_tensor("wv", [128, n_ct * 128], BF16, kind="ExternalInput")
    wp_d = nc.dram_tensor("wp", [128, C], BF16, kind="ExternalInput")
    tri_d = nc.dram_tensor("tri", [128, 128], BF16, kind="ExternalInput")
    idn_d = nc.dram_tensor("idn", [128, 128], BF16, kind="ExternalInput")
    idf_d = nc.dram_tensor("idf", [128, 128], F32, kind="ExternalInput")
    out_d = nc.dram_tensor("outp", [BT, C], F16, kind="ExternalOutput")

    with tile.TileContext(nc) as tc:
        with (
            tc.tile_pool(name="persist", bufs=1) as persist,
            tc.tile_pool(name="xt", bufs=2 * n_ct) as xt_pool,
            tc.tile_pool(name="pp", bufs=4) as p_pool,
            tc.tile_pool(name="vaug", bufs=2) as vaug_pool,
            tc.tile_pool(name="small", bufs=4) as small_pool,
            tc.tile_pool(name="rep", bufs=3) as rep_pool,
            tc.tile_pool(name="tmp1", bufs=3) as tmp_pool,
            tc.tile_pool(name="ps_mm", bufs=2, space="PSUM") as ps_mm,
            tc.tile_pool(name="ps_s", bufs=2, space="PSUM") as ps_s,
            tc.tile_pool(name="ps_yu0", bufs=1, space="PSUM") as ps_yu0,
            tc.tile_pool(name="ps_yu1", bufs=1, space="PSUM") as ps_yu1,
        ):
            # ---- persistent SBUF tensors -------------------------------
            qTs = [persist.tile([128, T], BF16, tag=f"qT{b}", name=f"qT{b}")
                   for b in range(B)]
            kTs = [persist.tile([128, T], BF16, tag=f"kT{b}", name=f"kT{b}")
                   for b in range(B)]
            vTs = [persist.tile([128, T], BF16, tag=f"vT{b}", name=f"vT{b}")
                   for b in range(B)]
            yuTs = [persist.tile([128, T], BF16, tag=f"yuT{b}", name=f"yuT{b}")
                    for b in range(B)]
            wq_sb = persist.tile([128, n_ct * 128], BF16, tag="wq")
            wk_sb = persist.tile([128, n_ct * 128], BF16, tag="wk")
            wv_sb = persist.tile([128, n_ct * 128], BF16, tag="wv")
            wp_sb = persist.tile([128, C], BF16, tag="wp")
            tri_sb = persist.tile([128, 128], BF16, tag="tri")
            idn_sb = persist.tile([128, 128], BF16, tag="idn")
            idf_sb = persist.tile([128, 128], F32, tag="idf")
            nc.sync.dma_start(idf_sb[:], idf_d[:])
            nc.sync.dma_start(wq_sb[:], wq_d[:])
            nc.sync.dma_start(wk_sb[:], wk_d[:])
            nc.sync.dma_start(wv_sb[:], wv_d[:])
            nc.sync.dma_start(wp_sb[:], wp_d[:])
            nc.sync.dma_start(tri_sb[:], tri_d[:])
            nc.sync.dma_start(idn_sb[:], idn_d[:])

            # ---- phase 1: qkv^T = W.T @ x^T ----------------------------
            for b in range(B):
                for gl in range(cfg.ngb):
                    g0 = b * T + gl * TG       # global col base
                    l0 = gl * TG               # batch-local col base
                    xts = []
                    for ci in range(n_ct):
                        xt = xt_pool.tile([128, TG], BF16, tag="xt")
                        nc.sync.dma_start(
                            xt[:], xT_d[128 * ci:128 * (ci + 1),
                                        g0:g0 + TG])
                        xts.append(xt)
                    for wsb, dsts in ((wq_sb, qTs), (wk_sb, kTs),
                                      (wv_sb, vTs)):
                        for ch in range(cfg.nchp):
                            ps = ps_mm.tile([128, CH], F32, tag="mm",
                                            name="ps")
                            for ci in range(n_ct):
                                nc.tensor.matmul(
                                    ps[:],
                                    wsb[:, 128 * ci:128 * (ci + 1)],
                                    xts[ci][:, ch * CH:(ch + 1) * CH],
                                    start=(ci == 0), stop=(ci == n_ct - 1))
                            nc.vector.tensor_copy(
                                dsts[b][:, l0 + ch * CH:l0 + (ch + 1) * CH],
                                ps[:])

            # ---- phases 2+3 per batch: v_aug, attention ----------------
            for b in range(B):
                qT, kT, vT, yuT = qTs[b], kTs[b], vTs[b], yuTs[b]
                # v natural layout + ones column, per head
                vaugs = []
                for h in (0, 1):
                    va = vaug_pool.tile([128, nt * 65], BF16, tag=f"vaug{h}",
                                        name=f"va{h}")
                    ones_cols = va[:].rearrange("p (i c) -> p i c",
                                                c=65)[:, :, 64]
                    nc.vector.memset(ones_cols, 1.0)
                    vaugs.append(va)
                for i in range(nt):
                    for h in (0, 1):
                        pt = ps_mm.tile([128, 64], BF16, tag="mm", name="pt")
                        nc.tensor.transpose(
                            pt[:],
                            vT[64 * h:64 * h + 64, 128 * i:128 * (i + 1)],
                            idn_sb[64 * h:64 * h + 64, 64 * h:64 * h + 64])
                        nc.vector.tensor_copy(
                            vaugs[h][:, 65 * i:65 * i + 64], pt[:])

                for j in range(ncw):
                    yub = [ps_yu0.tile([65, CH], F32, tag="yu0", name="yu0"),
                           ps_yu1.tile([65, CH], F32, tag="yu1", name="yu1")]
                    i_max = r * (j + 1) - 1
                    for i in range(r * (j + 1)):
                        c0 = 128 * (i - r * j) if i >= r * j else 0
                        w = CH - c0
                        ss = ps_s.tile([128, 2 * CH], F32, tag="s", name="ss")
                        for h in (0, 1):
                            nc.tensor.matmul(
                                ss[:, h * CH:h * CH + w],
                                kT[64 * h:64 * h + 64,
                                   128 * i:128 * (i + 1)],
                                qT[64 * h:64 * h + 64,
                                   CH * j + c0:CH * (j + 1)],
                                start=True, stop=True)
                        pt_ = p_pool.tile([128, 2 * w], BF16, tag="p",
                                          name="pt_")
                        if w == CH:
                            nc.scalar.activation(pt_[:], ss[:], AF.Exp,
                                                 scale=sm_scale)
                        else:
                            for h in (0, 1):
                                nc.scalar.activation(
                                    pt_[:, h * w:(h + 1) * w],
                                    ss[:, h * CH:h * CH + w], AF.Exp,
                                    scale=sm_scale)
                        if i >= r * j:
                            for h in (0, 1):
                                nc.vector.tensor_mul(
                                    pt_[:, h * w:h * w + 128],
                                    pt_[:, h * w:h * w + 128], tri_sb[:])
                        for h in (0, 1):
                            nc.tensor.matmul(
                                yub[h][:, c0:CH],
                                vaugs[h][:, 65 * i:65 * i + 65],
                                pt_[:, h * w:(h + 1) * w],
                                start=(i == 0), stop=(i == i_max))
                    # Copy yu out of PSUM right away so the accumulator
                    # banks free up for the next chunk, then normalize from
                    # SBUF: PE-transpose the denom rows to column layout,
                    # exact-recip on DVE (cheap on the free axis),
                    # PE-transpose back, broadcast across partitions.
                    yus = []
                    for h in (0, 1):
                        yc = small_pool.tile([65, CH], F32, tag=f"yus{h}",
                                             name=f"yus{h}")
                        nc.vector.tensor_copy(yc[:], yub[h][:])
                        yus.append(yc)
                    dt = ps_mm.tile([128, 2 * nq], F32, tag="mm", name="dt")
                    for h in (0, 1):
                        for k in range(nq):
                            nc.tensor.transpose(
                                dt[:, nq * h + k:nq * h + k + 1],
                                yus[h][64:65, 128 * k:128 * (k + 1)],
                                idf_sb[64:65, 64:65])
                    rcol = small_pool.tile([128, 2 * nq], F32, tag="rcol")
                    nc.vector.reciprocal(rcol[:], dt[:])
                    rb = ps_mm.tile([2 * nq, 128], F32, tag="mm", name="rb")
                    nc.tensor.transpose(rb[:], rcol[:], idf_sb[:, :])
                    rbs = small_pool.tile([2 * nq, 128], F32, tag="rbs")
                    nc.vector.tensor_copy(rbs[:], rb[:])
                    for h in (0, 1):
                        rec0 = small_pool.tile([1, CH], F32, tag="rec0",
                                               name="rec0")
                        nc.sync.dma_start(
                            rec0[0:1, :], rbs[nq * h:nq * (h + 1), :])
                        rep = rep_pool.tile([64, CH], F32, tag="rep",
                                            name="rep")
                        nc.gpsimd.partition_broadcast(rep[:], rec0[0:1, :])
                        cols = slice(CH * j, CH * (j + 1))
                        if h == 0:
                            nc.vector.tensor_mul(
                                yuT[0:64, cols], yus[0][0:64, :], rep[:])
                        else:
                            tm = tmp_pool.tile([64, CH], BF16, tag="tmp1")
                            nc.vector.tensor_mul(tm[:], yus[1][0:64, :],
                                                 rep[:])
                            nc.sync.dma_start(yuT[64:128, cols], tm[:])

            # ---- phase 4: out_p = y^T.T @ W_proj_rows ------------------
            for b in range(B):
                for m in range(nt):
                    po = ps_s.tile([128, 2 * CH], F32, tag="s", name="po")
                    for ch in range(C // CH):
                        nc.tensor.matmul(
                            po[:, ch * CH:(ch + 1) * CH],
                            yuTs[b][:, 128 * m:128 * (m + 1)],
                            wp_sb[:, ch * CH:(ch + 1) * CH],
                            start=True, stop=True)
                    ob = tmp_pool.tile([128, C], F16, tag="ob", bufs=4,
                                       name="ob")
                    if m % 2 == 0:
                        nc.scalar.copy(ob[:], po[:, 0:C])
                    else:
                        nc.vector.tensor_copy(ob[:], po[:, 0:C])
                    nc.sync.dma_start(
                        out_d[b * T + 128 * m:b * T + 128 * (m + 1), :],
                        ob[:])

    nc.compile()
    return nc


def host_inputs(cfg: Cfg, x, W_attn, W_proj, h0, hpc=2):
    """Per-core input dict for the core owning heads [h0, h0+hpc)."""
    C, D = cfg.C, cfg.D
    assert hpc * D == 128
    bf = ml_dtypes.bfloat16

    def wpack(Wcols):  # [C, 128] -> [128, n_ct*128] (c-tile minor)
        return np.ascontiguousarray(
            Wcols.reshape(cfg.n_ct, 128, 128).transpose(1, 0, 2)
            .reshape(128, cfg.n_ct * 128)).astype(bf)

    cols = np.concatenate([np.arange(h * D, (h + 1) * D)
                           for h in range(h0, h0 + hpc)])
    return {
        "wq": wpack(W_attn[:, cols]),
        "wk": wpack(W_attn[:, C + cols]),
        "wv": wpack(W_attn[:, 2 * C + cols]),
        "wp": np.ascontiguousarray(W_proj[cols, :]).astype(bf),
        "tri": np.triu(np.ones((128, 128))).astype(bf),
        "idn": np.eye(128).astype(bf),
        "idf": np.eye(128, dtype=np.float32),
    }


_NC_CACHE = {}


def kernel(x, W_attn, W_proj):
    x = np.asarray(x)
    W_attn = np.asarray(W_attn)
    W_proj = np.asarray(W_proj)
    B, T, C = x.shape
    cfg = Cfg(B=B, T=T, C=C)
    key = (B, T, C)
    if key not in _NC_CACHE:
        _NC_CACHE[key] = build(cfg)
    nc = _NC_CACHE[key]

    xT = np.ascontiguousarray(x.reshape(cfg.BT, C).T).astype(ml_dtypes.bfloat16)
    in_maps = []
    for core in range(N_CORES):
        m = host_inputs(cfg, x, W_attn, W_proj, h0=2 * core)
        m["xT"] = xT
        in_maps.append(m)

    res = run_bass_kernel_spmd(nc, in_maps, core_ids=list(range(N_CORES)))
    out = np.zeros((cfg.BT, C), dtype=np.float64)
    for core in range(N_CORES):
        out += res.results[core]["outp"].astype(np.float64)
    return out.astype(np.float32).reshape(B, T, C)
